# revision 9
# baseline (speedup 1.0000x reference)
# Trainium2 Bass kernel for nn_ConceptEncodingBlock (B=4, L=512, M=32, EMB=512, H=8).
#
# Math restructure (exact, linearity of the slot projection):
#   reference:  v_ = einsum('mwv,blv->bmlw', v, h)  (34.4 GFLOP)
#               out = einsum('bhml,bmlhs->bmhs', softmax(q cells), v_)
#   here:       c[b,m,h,:] = sum_l attn[b,h,m,l] * h[b,l,:]      (0.54 GFLOP)
#               out[b,m,h,s] = sum_e c[b,m,h,e] * v[m,h*HS+s,e] + vb[m,h*HS+s]
#   (sum_l attn == 1 exactly in softmax, so the vb term is a constant add)
#
# The layernorm runs on the HOST (microseconds of numpy): the device receives
# xh = (x-mu)*rstd in bf16, in both layouts (l-major for the weighted average,
# e-major for the scores). That removes bn_stats/sqrt/rstd machinery entirely:
#   - scores: k'[mh,e] = cells-row @ q_w (q projection + ln_g + 1/sqrt(HS)
#     folded on host; q_b/ln_b cancel in the softmax), one matmul chain per
#     batch over xh^T; exp needs no per-partition scale -> one exp per batch.
#   - weighted avg: cu[mh,e] = sum_l exp[l,mh] xh[l,e]; the denominator
#     sum_l exp comes from an extra all-ones column appended to xh (col 512),
#     contracted in tiny side matmuls; c = cu * (1/den).
#   - out: o_j[(b,h),w] = sum_e c[e,(b,h)] vT[j][e,w] + vb  (vT bf16).
#
# Perf structure (trace-driven):
#   - all big operands bf16: 6.1MB input DMA at the ~360GB/s DMA roofline.
#     Six >=1MB DMAs on the sync queue in consumption order (xh^T halves,
#     xh halves, vT halves) — small DMAs bleed ~0.5us each in issue gaps.
#   - scores/exp complete while xh/vT still stream; the only post-DMA tail is
#     M3 on the last vT half plus the vb add.
#   - single act-table load (exp), no sqrt anywhere.
#
# Sharding: slot dim m split 4-per-core over 8 cores; full batch per core.

import ml_dtypes
import numpy as np

import concourse.bass as bass
import concourse.mybir as mybir
import concourse.tile as tile
from concourse.bass_utils import run_bass_kernel_spmd

B, L, M, EMB, H = 4, 512, 32, 512, 8
HS = EMB // H          # 64
LN_EPS = 1e-5
N_CORES = 8
S = M // N_CORES       # 4 slots per core
MH = H * S             # 32 (h, slot) pairs per core; mh = h*S + j
F32 = mybir.dt.float32
F32R = mybir.dt.float32r
BF16 = mybir.dt.bfloat16
SCALE = float(HS) ** -0.5  # 0.125 (folded into the host key matrix)
BL = B * L
XC = EMB + 1           # xh free width: 512 data cols + ones col
BF = ml_dtypes.bfloat16


def _split_excess_waits(nc, limit=1):
    """walrus in this container accepts only 1 embedded sync-wait per
    instruction (CTRL and the matmul LDWEIGHTS side both overflow at 2);
    hoist excess waits onto inserted same-engine NoOp carriers (sequential
    waits are semantically identical to combined waits)."""
    n = 0
    for f in nc.m.functions:
        for bb in f.blocks:
            insts = bb.instructions
            i = 0
            while i < len(insts):
                ins = insts[i]
                si = ins.sync_info
                if si is not None and si.on_wait and len(si.on_wait) > limit:
                    waits = list(si.on_wait)
                    keep, rest = waits[:limit], waits[limit:]
                    carriers = []
                    for k in range(len(rest)):
                        n += 1
                        carriers.append(
                            mybir.InstNoOp(
                                name=f"wait-split-{n}",
                                engine=ins.engine,
                                ins=[],
                                outs=[],
                                sync_info=mybir.SyncInfo(
                                    on_wait=rest[k : k + 1], on_update=[]
                                ),
                            )
                        )
                    ins.sync_info = mybir.SyncInfo(
                        on_wait=keep, on_update=list(si.on_update)
                    )
                    for k, c in enumerate(carriers):
                        insts.insert(i + k, c)
                    i += len(carriers)
                i += 1
    return n


def _build_nc():
    nc = bass.Bass()
    xh_d = nc.dram_tensor("xhd", [128, B, 4, XC], BF16, kind="ExternalInput")
    xt_d = nc.dram_tensor("xtd", [128, B, 4, L], BF16, kind="ExternalInput")
    kt_d = nc.dram_tensor("ktd", [128, 4, MH], BF16, kind="ExternalInput")
    vt_d = nc.dram_tensor("vtd", [128, S, 4, EMB], BF16, kind="ExternalInput")
    vb_d = nc.dram_tensor("vbd", [1, S, EMB], F32, kind="ExternalInput")
    idr_d = nc.dram_tensor("idrd", [32, 32], F32, kind="ExternalInput")
    out_d = nc.dram_tensor("out", [S, 32, EMB], F32, kind="ExternalOutput")

    with tile.TileContext(nc) as tc:
        with (
            tc.tile_pool(name="big", bufs=1) as big,
            tc.tile_pool(name="small", bufs=1) as small,
            tc.tile_pool(name="work", bufs=2) as work,
            tc.tile_pool(name="ps", bufs=1, space="PSUM") as ps,
        ):
            # persistent tensors
            xh_sb = big.tile([128, B, 4, XC], BF16)     # xhat | ones; rows l%128
            xT_sb = big.tile([128, B, 4, L], BF16)      # xhat^T; rows e%128
            vT_sb = big.tile([128, S, 4, EMB], BF16)    # (j, ec, w)
            kT_sb = small.tile([128, 4, MH], BF16)      # 0.125 * keys (ec, mh)
            vb_bc = small.tile([32, S, EMB], F32)       # vb broadcast over partitions
            ident_r = small.tile([32, 32], F32R)
            cT = small.tile([128, EMB], BF16)           # (ec, b, mh); rows e%128

            # ---- small input DMAs on the gpsimd (SWDGE) queue
            nc.gpsimd.dma_start(out=kT_sb, in_=kt_d[:, :, :])
            nc.gpsimd.dma_start(out=ident_r, in_=idr_d[:, :].bitcast(F32R))
            for j in range(S):
                nc.gpsimd.dma_start(
                    out=vb_bc[:, j, :],
                    in_=vb_d[0:1, j, :].partition_broadcast(32),
                )

            # ---- big input DMAs: one sync-queue stream, >=1MB each, in
            # consumption order: scores need xh^T first, then xh, then vT.
            nc.sync.dma_start(out=xT_sb[:, 0:2, :, :], in_=xt_d[:, 0:2, :, :])
            nc.sync.dma_start(out=xh_sb[:, 0:2, :, :], in_=xh_d[:, 0:2, :, :])
            nc.sync.dma_start(out=xT_sb[:, 2:4, :, :], in_=xt_d[:, 2:4, :, :])
            nc.sync.dma_start(out=xh_sb[:, 2:4, :, :], in_=xh_d[:, 2:4, :, :])
            nc.sync.dma_start(out=vT_sb[:, 0:2, :, :], in_=vt_d[:, 0:2, :, :])
            nc.sync.dma_start(out=vT_sb[:, 2:4, :, :], in_=vt_d[:, 2:4, :, :])

            # ---- PE warmup: ~28 dummy matmuls on the key tile ramp the
            # tensor engine to full p-state (2.4GHz) during the DMA prefix,
            # so the real matmul stream runs at 213ns/512col, not 630ns.
            warm_ps = ps.tile([32, EMB], F32, tag="cu", bufs=1)
            for w in range(28):
                nc.tensor.matmul(
                    warm_ps[:, 0:128],
                    kT_sb[:, w % 4, :],
                    kT_sb.rearrange("p a c -> p (a c)"),
                    start=True, stop=True,
                )

            # ---- scores: M1 for a batch pair as soon as its xh^T half lands;
            # PSUM->SBUF copies ride the idle vector engine; transposes and
            # the single per-batch exp follow.
            rawcs, expTs = [], []
            def m1_pair(b0):
                for b in (b0, b0 + 1):
                    rawc_ps = ps.tile([32, L], F32, tag="rawc", bufs=2)
                    for ec in range(4):
                        nc.tensor.matmul(
                            rawc_ps,
                            kT_sb[:, ec, :],
                            xT_sb[:, b, ec, :],
                            start=(ec == 0), stop=(ec == 3),
                        )
                    rawc_sb = work.tile([32, 4, 128], F32R, tag="rawc_sb")
                    nc.vector.tensor_copy(out=rawc_sb, in_=rawc_ps.bitcast(F32R))
                    rawcs.append(rawc_sb)

            def score_pair(b0):
                for b in (b0, b0 + 1):
                    sct_ps = ps.tile([128, 4, MH], F32R, tag="sct", bufs=2)
                    for lc in range(4):
                        nc.tensor.transpose(
                            out=sct_ps[:, lc, :],
                            in_=rawcs[b][:, lc, :],
                            identity=ident_r,
                        )
                    expT = work.tile([128, 4, MH], BF16, tag="expT", bufs=4)
                    nc.scalar.activation(
                        out=expT, in_=sct_ps,
                        func=mybir.ActivationFunctionType.Exp,
                        bias=0.0, scale=1.0,
                    )
                    expTs.append(expT)

            # ---- weighted average for one batch (needs xh + expT)
            c_bs, rcs = [], []
            def avg_batch(b):
                expT = expTs[b]
                dns_ps = ps.tile([32, EMB], F32, tag="oj", bufs=2)
                for lc in range(4):
                    nc.tensor.matmul(
                        dns_ps[:, 0:1],
                        expT[:, lc, :],
                        xh_sb[:, b, lc, EMB : EMB + 1],
                        start=(lc == 0), stop=(lc == 3),
                    )
                rc_b = work.tile([32, 1], F32, tag="rc_b", bufs=4)
                nc.vector.reciprocal(out=rc_b, in_=dns_ps[:, 0:1])
                rcs.append(rc_b)

                cu_ps = ps.tile([32, EMB], F32, tag="cu", bufs=1)
                for lc in range(4):
                    nc.tensor.matmul(
                        cu_ps,
                        expT[:, lc, :],
                        xh_sb[:, b, lc, 0:EMB],
                        start=(lc == 0), stop=(lc == 3),
                    )
                c_b = work.tile([32, 4, 128], F32R, tag="c_b", bufs=4)
                nc.vector.tensor_scalar_mul(
                    out=c_b, in0=cu_ps.bitcast(F32R), scalar1=rc_b
                )
                c_bs.append(c_b)

            m1_pair(0)
            score_pair(0)
            avg_batch(0)
            avg_batch(1)
            m1_pair(2)
            score_pair(2)
            avg_batch(2)
            avg_batch(3)

            ct_ps = ps.tile([128, EMB], F32R, tag="ct", bufs=1)
            for b in range(B):
                for ec in range(4):
                    nc.tensor.transpose(
                        out=ct_ps[:, ec * 128 + b * 32 : ec * 128 + b * 32 + 32],
                        in_=c_bs[b][:, ec, :],
                        identity=ident_r,
                    )
            nc.scalar.copy(out=cT, in_=ct_ps)
            cT_v = cT.rearrange("p (ec b h j) -> p ec b h j", ec=4, b=B, h=H, j=S)

            # M3 (bf16): o_j[(b,h), w] = sum_e c[(b,h*S+j), e] vT[j][e, w] + vb
            for j in range(S):
                oj_ps = ps.tile([32, EMB], F32, tag="oj", bufs=2)
                for ec in range(4):
                    nc.tensor.matmul(
                        oj_ps,
                        cT_v[:, ec, :, :, j],
                        vT_sb[:, j, ec, :],
                        start=(ec == 0), stop=(ec == 3),
                    )
                oj_sb = work.tile([32, EMB], F32, tag="oj_sb")
                nc.vector.tensor_add(out=oj_sb, in0=oj_ps, in1=vb_bc[:, j, :])
                nc.gpsimd.dma_start(out=out_d[j, :, :], in_=oj_sb)

    _split_excess_waits(nc)
    return nc


_NC_CACHE = {}


def _get_nc():
    if "nc" not in _NC_CACHE:
        _NC_CACHE["nc"] = _build_nc()
    return _NC_CACHE["nc"]


def _prepare_in_maps(x, cells, q_w, q_b, v, vb, ln_g, ln_b):
    x2d = np.ascontiguousarray(x.reshape(BL, EMB), dtype=np.float32)
    # host layernorm (no affine; ln_g/ln_b are folded into the weights)
    mu = x2d.mean(axis=1, keepdims=True)
    var = x2d.var(axis=1, keepdims=True)
    xh = (x2d - mu) / np.sqrt(var + LN_EPS)

    # xhd: [p, b, lc, 513] with l = lc*128 + p; col 512 == 1.0
    xh_aug = np.ones((BL, XC), dtype=np.float32)
    xh_aug[:, :EMB] = xh
    xhd_host = np.ascontiguousarray(
        xh_aug.reshape(B, 4, 128, XC).transpose(2, 0, 1, 3)
    ).astype(BF)
    # xtd: [p, b, ec, l] with e = ec*128 + p
    xt3 = xh.T.reshape(4, 128, B, L)                # [ec, p, b, l]
    xtd_host = np.ascontiguousarray(xt3.transpose(1, 2, 0, 3)).astype(BF)

    ln_g = ln_g.astype(np.float32)
    q_w_eff = (q_w * ln_g[None, :]).astype(np.float32)      # fold g into keys
    idr = np.eye(32, dtype=np.float32)

    in_maps = []
    for core in range(N_CORES):
        m0 = core * S
        # k'[mh, e] with mh = h*S + j; fold in the 1/sqrt(HS) score scale.
        kp = np.zeros((MH, EMB), dtype=np.float32)
        for h in range(H):
            wslice = slice(h * HS, (h + 1) * HS)
            for j in range(S):
                c_hj = cells[m0 + j, h, :].astype(np.float32)
                kp[h * S + j] = c_hj @ q_w_eff[wslice, :]
        kp -= kp.mean(axis=1, keepdims=True)
        kp *= SCALE
        # ktd: [p, ec, mh] with e = ec*128 + p
        ktd_host = np.ascontiguousarray(
            kp.T.reshape(4, 128, MH).transpose(1, 0, 2)
        ).astype(BF)

        vslab = v[m0 : m0 + S].astype(np.float32)            # (S, EMB, EMB) [j, w, e]
        vT = vslab.transpose(0, 2, 1) * ln_g[None, :, None]  # (S, e, w), g folded
        # vtd: [p, j, ec, w] with e = ec*128 + p
        vtd_host = np.ascontiguousarray(
            vT.reshape(S, 4, 128, EMB).transpose(2, 0, 1, 3)
        ).astype(BF)
        vb_host = (
            vb[m0 : m0 + S] + vslab @ ln_b.astype(np.float32)
        ).astype(np.float32).reshape(1, S, EMB)

        in_maps.append(
            {
                "xhd": xhd_host,
                "xtd": xtd_host,
                "ktd": ktd_host,
                "vtd": vtd_host,
                "vbd": np.ascontiguousarray(vb_host),
                "idrd": idr,
            }
        )
    return in_maps


def _assemble(results):
    out_pre = np.empty((B, M, H, HS), dtype=np.float32)
    for core in range(N_CORES):
        m0 = core * S
        o = results[core]["out"]                    # (S, 32, 512) rows (b,h)
        o5 = o.reshape(S, B, H, H, HS)              # [j, b, h, h', s]
        out_pre[:, m0 : m0 + S] = np.einsum("jbhhs->bjhs", o5)
    # faithful to torch: transpose(1,2) then reshape(-1, m, emb)
    return np.ascontiguousarray(
        np.swapaxes(out_pre, 1, 2).reshape(B, M, EMB)
    ).astype(np.float32)


def kernel(x, cells, q_w, q_b, v, vb, ln_g, ln_b, _trace=False):
    x = np.asarray(x, dtype=np.float32)
    cells = np.asarray(cells, dtype=np.float32)
    q_w = np.asarray(q_w, dtype=np.float32)
    q_b = np.asarray(q_b, dtype=np.float32)
    v = np.asarray(v, dtype=np.float32)
    vb = np.asarray(vb, dtype=np.float32)
    ln_g = np.asarray(ln_g, dtype=np.float32)
    ln_b = np.asarray(ln_b, dtype=np.float32)
    nc = _get_nc()
    in_maps = _prepare_in_maps(x, cells, q_w, q_b, v, vb, ln_g, ln_b)
    res = run_bass_kernel_spmd(nc, in_maps, core_ids=list(range(N_CORES)), trace=_trace)
    out = _assemble(res.results)
    if _trace:
        return out, res
    return out


# revision 11
# speedup vs baseline: 1.0014x; 1.0014x over previous
# Trainium2 Bass kernel for nn_ConceptEncodingBlock (B=4, L=512, M=32, EMB=512, H=8).
#
# Math restructure (exact, linearity of the slot projection):
#   reference:  v_ = einsum('mwv,blv->bmlw', v, h)  (34.4 GFLOP)
#               out = einsum('bhml,bmlhs->bmhs', softmax(q cells), v_)
#   here:       c[b,m,h,:] = sum_l attn[b,h,m,l] * h[b,l,:]      (0.54 GFLOP)
#               out[b,m,h,s] = sum_e c[b,m,h,e] * v[m,h*HS+s,e] + vb[m,h*HS+s]
#   (sum_l attn == 1 exactly in softmax, so the vb term is a constant add)
#
# The layernorm runs on the HOST (microseconds of numpy): the device receives
# xh = (x-mu)*rstd in bf16, in both layouts (l-major for the weighted average,
# e-major for the scores). That removes bn_stats/sqrt/rstd machinery entirely:
#   - scores: k'[mh,e] = cells-row @ q_w (q projection + ln_g + 1/sqrt(HS)
#     folded on host; q_b/ln_b cancel in the softmax), one matmul chain per
#     batch over xh^T; exp needs no per-partition scale -> one exp per batch.
#   - weighted avg: cu[mh,e] = sum_l exp[l,mh] xh[l,e]; the denominator
#     sum_l exp comes from an extra all-ones column appended to xh (col 512),
#     contracted in tiny side matmuls; c = cu * (1/den).
#   - out: o_j[(b,h),w] = sum_e c[e,(b,h)] vT[j][e,w] + vb  (vT bf16).
#
# Perf structure (trace-driven):
#   - all big operands bf16: 6.1MB input DMA at the ~360GB/s DMA roofline.
#     Six >=1MB DMAs on the sync queue in consumption order (xh^T halves,
#     xh halves, vT halves) — small DMAs bleed ~0.5us each in issue gaps.
#   - scores/exp complete while xh/vT still stream; the only post-DMA tail is
#     M3 on the last vT half plus the vb add.
#   - single act-table load (exp), no sqrt anywhere.
#
# Sharding: slot dim m split 4-per-core over 8 cores; full batch per core.

import ml_dtypes
import numpy as np

import concourse.bass as bass
import concourse.mybir as mybir
import concourse.tile as tile
from concourse.bass_utils import run_bass_kernel_spmd

B, L, M, EMB, H = 4, 512, 32, 512, 8
HS = EMB // H          # 64
LN_EPS = 1e-5
N_CORES = 8
S = M // N_CORES       # 4 slots per core
MH = H * S             # 32 (h, slot) pairs per core; mh = h*S + j
F32 = mybir.dt.float32
F32R = mybir.dt.float32r
BF16 = mybir.dt.bfloat16
SCALE = float(HS) ** -0.5  # 0.125 (folded into the host key matrix)
BL = B * L
XC = EMB + 1           # xh free width: 512 data cols + ones col
BF = ml_dtypes.bfloat16


def _split_excess_waits(nc, limit=1):
    """walrus in this container accepts only 1 embedded sync-wait per
    instruction (CTRL and the matmul LDWEIGHTS side both overflow at 2);
    hoist excess waits onto inserted same-engine NoOp carriers (sequential
    waits are semantically identical to combined waits)."""
    n = 0
    for f in nc.m.functions:
        for bb in f.blocks:
            insts = bb.instructions
            i = 0
            while i < len(insts):
                ins = insts[i]
                si = ins.sync_info
                if si is not None and si.on_wait and len(si.on_wait) > limit:
                    waits = list(si.on_wait)
                    keep, rest = waits[:limit], waits[limit:]
                    carriers = []
                    for k in range(len(rest)):
                        n += 1
                        carriers.append(
                            mybir.InstNoOp(
                                name=f"wait-split-{n}",
                                engine=ins.engine,
                                ins=[],
                                outs=[],
                                sync_info=mybir.SyncInfo(
                                    on_wait=rest[k : k + 1], on_update=[]
                                ),
                            )
                        )
                    ins.sync_info = mybir.SyncInfo(
                        on_wait=keep, on_update=list(si.on_update)
                    )
                    for k, c in enumerate(carriers):
                        insts.insert(i + k, c)
                    i += len(carriers)
                i += 1
    return n


def _build_nc():
    nc = bass.Bass()
    xh_d = nc.dram_tensor("xhd", [128, B, 4, XC], BF16, kind="ExternalInput")
    xt_d = nc.dram_tensor("xtd", [128, B, 4, L], BF16, kind="ExternalInput")
    kt_d = nc.dram_tensor("ktd", [128, 4, MH], BF16, kind="ExternalInput")
    vt_d = nc.dram_tensor("vtd", [128, S, 4, EMB], BF16, kind="ExternalInput")
    vb_d = nc.dram_tensor("vbd", [1, S, EMB], F32, kind="ExternalInput")
    idr_d = nc.dram_tensor("idrd", [32, 32], F32, kind="ExternalInput")
    out_d = nc.dram_tensor("out", [S, 32, EMB], F32, kind="ExternalOutput")

    with tile.TileContext(nc) as tc:
        with (
            tc.tile_pool(name="big", bufs=1) as big,
            tc.tile_pool(name="small", bufs=1) as small,
            tc.tile_pool(name="work", bufs=2) as work,
            tc.tile_pool(name="avg", bufs=2) as avg,
            tc.tile_pool(name="ps", bufs=1, space="PSUM") as ps,
        ):
            # persistent tensors
            xh_sb = big.tile([128, B, 4, XC], BF16)     # xhat | ones; rows l%128
            xT_sb = big.tile([128, B, 4, L], BF16)      # xhat^T; rows e%128
            vT_sb = big.tile([128, S, 4, EMB], BF16)    # (j, ec, w)
            kT_sb = small.tile([128, 4, MH], BF16)      # 0.125 * keys (ec, mh)
            vb_bc = small.tile([32, S, EMB], F32)       # vb broadcast over partitions
            ident_r = small.tile([32, 32], F32R)
            cT = small.tile([128, EMB], BF16)           # (ec, b, mh); rows e%128

            # ---- keys + identity first on the fast HWDGE queue (the PE
            # warmup below needs kT as early as possible); vb via gpsimd
            nc.sync.dma_start(out=kT_sb, in_=kt_d[:, :, :])
            nc.sync.dma_start(out=ident_r, in_=idr_d[:, :].bitcast(F32R))
            for j in range(S):
                nc.gpsimd.dma_start(
                    out=vb_bc[:, j, :],
                    in_=vb_d[0:1, j, :].partition_broadcast(32),
                )

            # ---- big input DMAs: one sync-queue stream, >=1MB each, in
            # consumption order: scores need xh^T first, then xh, then vT.
            nc.sync.dma_start(out=xT_sb[:, 0:2, :, :], in_=xt_d[:, 0:2, :, :])
            nc.sync.dma_start(out=xh_sb[:, 0:2, :, :], in_=xh_d[:, 0:2, :, :])
            nc.sync.dma_start(out=xT_sb[:, 2:4, :, :], in_=xt_d[:, 2:4, :, :])
            nc.sync.dma_start(out=xh_sb[:, 2:4, :, :], in_=xh_d[:, 2:4, :, :])
            nc.sync.dma_start(out=vT_sb[:, 0:2, :, :], in_=vt_d[:, 0:2, :, :])
            nc.sync.dma_start(out=vT_sb[:, 2:4, :, :], in_=vt_d[:, 2:4, :, :])

            # ---- PE warmup: ~28 dummy matmuls on the key tile ramp the
            # tensor engine to full p-state (2.4GHz) during the DMA prefix,
            # so the real matmul stream runs at 213ns/512col, not 630ns.
            warm_ps = ps.tile([32, EMB], F32, tag="cu", bufs=1)
            for w in range(34):
                nc.tensor.matmul(
                    warm_ps[:, 0:128],
                    kT_sb[:, w % 4, :],
                    kT_sb.rearrange("p a c -> p (a c)"),
                    start=True, stop=True,
                )

            # ---- scores: M1 for a batch pair as soon as its xh^T half lands;
            # PSUM->SBUF copies ride the idle vector engine; transposes and
            # the single per-batch exp follow.
            rawcs, expTs = [], []
            def m1_pair(b0):
                for b in (b0, b0 + 1):
                    rawc_ps = ps.tile([32, L], F32, tag="rawc", bufs=2)
                    for ec in range(4):
                        nc.tensor.matmul(
                            rawc_ps,
                            kT_sb[:, ec, :],
                            xT_sb[:, b, ec, :],
                            start=(ec == 0), stop=(ec == 3),
                        )
                    rawc_sb = work.tile([32, 4, 128], F32R, tag="rawc_sb")
                    nc.vector.tensor_copy(out=rawc_sb, in_=rawc_ps.bitcast(F32R))
                    rawcs.append(rawc_sb)

            def score_pair(b0):
                for b in (b0, b0 + 1):
                    sct_ps = ps.tile([128, 4, MH], F32R, tag="sct", bufs=2)
                    for lc in range(4):
                        nc.tensor.transpose(
                            out=sct_ps[:, lc, :],
                            in_=rawcs[b][:, lc, :],
                            identity=ident_r,
                        )
                    expT = work.tile([128, 4, MH], BF16, tag="expT", bufs=4)
                    nc.scalar.activation(
                        out=expT, in_=sct_ps,
                        func=mybir.ActivationFunctionType.Exp,
                        bias=0.0, scale=1.0,
                    )
                    expTs.append(expT)

            # ---- weighted average for one batch (needs xh + expT)
            c_bs, rcs = [], []
            def avg_batch(b):
                expT = expTs[b]
                dns_ps = ps.tile([32, EMB], F32, tag="oj", bufs=2)
                for lc in range(4):
                    nc.tensor.matmul(
                        dns_ps[:, 0:1],
                        expT[:, lc, :],
                        xh_sb[:, b, lc, EMB : EMB + 1],
                        start=(lc == 0), stop=(lc == 3),
                    )
                rc_b = avg.tile([32, 1], F32, tag="rc_b", bufs=4)
                nc.vector.reciprocal(out=rc_b, in_=dns_ps[:, 0:1])
                rcs.append(rc_b)

                cu_ps = ps.tile([32, EMB], F32, tag="cu", bufs=1)
                for lc in range(4):
                    nc.tensor.matmul(
                        cu_ps,
                        expT[:, lc, :],
                        xh_sb[:, b, lc, 0:EMB],
                        start=(lc == 0), stop=(lc == 3),
                    )
                c_b = avg.tile([32, 4, 128], F32R, tag="c_b", bufs=4)
                nc.vector.tensor_scalar_mul(
                    out=c_b, in0=cu_ps.bitcast(F32R), scalar1=rc_b
                )
                c_bs.append(c_b)

            m1_pair(0)
            score_pair(0)
            avg_batch(0)
            avg_batch(1)
            m1_pair(2)
            score_pair(2)
            avg_batch(2)
            avg_batch(3)

            # ec-major transposes with per-ec PSUM->SBUF copies so M3's
            # ec-chunk matmuls start before the full cT barrier resolves.
            ct_ps = ps.tile([128, EMB], F32R, tag="ct", bufs=1)
            for ec in range(4):
                for b in range(B):
                    nc.tensor.transpose(
                        out=ct_ps[:, ec * 128 + b * 32 : ec * 128 + b * 32 + 32],
                        in_=c_bs[b][:, ec, :],
                        identity=ident_r,
                    )
                nc.vector.tensor_copy(
                    out=cT[:, ec * 128 : (ec + 1) * 128],
                    in_=ct_ps[:, ec * 128 : (ec + 1) * 128],
                )
            cT_v = cT.rearrange("p (ec b h j) -> p ec b h j", ec=4, b=B, h=H, j=S)

            # keep the PE hot across the vT-wait so M3 runs at full p-state
            for w in range(14):
                nc.tensor.matmul(
                    warm_ps[:, 0:128],
                    kT_sb[:, w % 4, :],
                    kT_sb.rearrange("p a c -> p (a c)"),
                    start=True, stop=True,
                )

            # M3 (bf16): o_j[(b,h), w] = sum_e c[(b,h*S+j), e] vT[j][e, w] + vb
            for j in range(S):
                oj_ps = ps.tile([32, EMB], F32, tag="oj", bufs=2)
                for ec in range(4):
                    nc.tensor.matmul(
                        oj_ps,
                        cT_v[:, ec, :, :, j],
                        vT_sb[:, j, ec, :],
                        start=(ec == 0), stop=(ec == 3),
                    )
                oj_sb = avg.tile([32, EMB], F32, tag="oj_sb")
                nc.vector.tensor_add(out=oj_sb, in0=oj_ps, in1=vb_bc[:, j, :])
                nc.gpsimd.dma_start(out=out_d[j, :, :], in_=oj_sb)

    _split_excess_waits(nc)
    return nc


_NC_CACHE = {}


def _get_nc():
    if "nc" not in _NC_CACHE:
        _NC_CACHE["nc"] = _build_nc()
    return _NC_CACHE["nc"]


def _prepare_in_maps(x, cells, q_w, q_b, v, vb, ln_g, ln_b):
    x2d = np.ascontiguousarray(x.reshape(BL, EMB), dtype=np.float32)
    # host layernorm (no affine; ln_g/ln_b are folded into the weights)
    mu = x2d.mean(axis=1, keepdims=True)
    var = x2d.var(axis=1, keepdims=True)
    xh = (x2d - mu) / np.sqrt(var + LN_EPS)

    # xhd: [p, b, lc, 513] with l = lc*128 + p; col 512 == 1.0
    xh_aug = np.ones((BL, XC), dtype=np.float32)
    xh_aug[:, :EMB] = xh
    xhd_host = np.ascontiguousarray(
        xh_aug.reshape(B, 4, 128, XC).transpose(2, 0, 1, 3)
    ).astype(BF)
    # xtd: [p, b, ec, l] with e = ec*128 + p
    xt3 = xh.T.reshape(4, 128, B, L)                # [ec, p, b, l]
    xtd_host = np.ascontiguousarray(xt3.transpose(1, 2, 0, 3)).astype(BF)

    ln_g = ln_g.astype(np.float32)
    q_w_eff = (q_w * ln_g[None, :]).astype(np.float32)      # fold g into keys
    idr = np.eye(32, dtype=np.float32)

    in_maps = []
    for core in range(N_CORES):
        m0 = core * S
        # k'[mh, e] with mh = h*S + j; fold in the 1/sqrt(HS) score scale.
        kp = np.zeros((MH, EMB), dtype=np.float32)
        for h in range(H):
            wslice = slice(h * HS, (h + 1) * HS)
            for j in range(S):
                c_hj = cells[m0 + j, h, :].astype(np.float32)
                kp[h * S + j] = c_hj @ q_w_eff[wslice, :]
        kp -= kp.mean(axis=1, keepdims=True)
        kp *= SCALE
        # ktd: [p, ec, mh] with e = ec*128 + p
        ktd_host = np.ascontiguousarray(
            kp.T.reshape(4, 128, MH).transpose(1, 0, 2)
        ).astype(BF)

        vslab = v[m0 : m0 + S].astype(np.float32)            # (S, EMB, EMB) [j, w, e]
        vT = vslab.transpose(0, 2, 1) * ln_g[None, :, None]  # (S, e, w), g folded
        # vtd: [p, j, ec, w] with e = ec*128 + p
        vtd_host = np.ascontiguousarray(
            vT.reshape(S, 4, 128, EMB).transpose(2, 0, 1, 3)
        ).astype(BF)
        vb_host = (
            vb[m0 : m0 + S] + vslab @ ln_b.astype(np.float32)
        ).astype(np.float32).reshape(1, S, EMB)

        in_maps.append(
            {
                "xhd": xhd_host,
                "xtd": xtd_host,
                "ktd": ktd_host,
                "vtd": vtd_host,
                "vbd": np.ascontiguousarray(vb_host),
                "idrd": idr,
            }
        )
    return in_maps


def _assemble(results):
    out_pre = np.empty((B, M, H, HS), dtype=np.float32)
    for core in range(N_CORES):
        m0 = core * S
        o = results[core]["out"]                    # (S, 32, 512) rows (b,h)
        o5 = o.reshape(S, B, H, H, HS)              # [j, b, h, h', s]
        out_pre[:, m0 : m0 + S] = np.einsum("jbhhs->bjhs", o5)
    # faithful to torch: transpose(1,2) then reshape(-1, m, emb)
    return np.ascontiguousarray(
        np.swapaxes(out_pre, 1, 2).reshape(B, M, EMB)
    ).astype(np.float32)


def kernel(x, cells, q_w, q_b, v, vb, ln_g, ln_b, _trace=False):
    x = np.asarray(x, dtype=np.float32)
    cells = np.asarray(cells, dtype=np.float32)
    q_w = np.asarray(q_w, dtype=np.float32)
    q_b = np.asarray(q_b, dtype=np.float32)
    v = np.asarray(v, dtype=np.float32)
    vb = np.asarray(vb, dtype=np.float32)
    ln_g = np.asarray(ln_g, dtype=np.float32)
    ln_b = np.asarray(ln_b, dtype=np.float32)
    nc = _get_nc()
    in_maps = _prepare_in_maps(x, cells, q_w, q_b, v, vb, ln_g, ln_b)
    res = run_bass_kernel_spmd(nc, in_maps, core_ids=list(range(N_CORES)), trace=_trace)
    out = _assemble(res.results)
    if _trace:
        return out, res
    return out
